# revision 71
# baseline (speedup 1.0000x reference)
"""Trainium2 Bass kernel for nn_FANPhaseOffsetTransformerLayer.

Full inputs -> full output. Sharding: 8 cores; core c handles batch b=c//4
and sequence-row chunk qc=c%4 (512 rows) of that batch. Each core computes
k/v for its whole batch (redundant but used as PE filler inside the
exp-bound attention window), q only for its row chunk, attention for its
rows over all 16 heads, then Wo/LN1/FAN/LN2 for its rows.

v2 changes vs baseline:
- PE warmup burst at t=0 (HAM clock-gate released before real matmuls).
- x^T is pre-cast to fp8 on the host (extra input) - kills the in-kernel
  DVE casts and the v-fill dependency on them.
- DMA queue assignment/order tuned so q-projection and quarter-0 k/v
  inputs land first; quarter-0 fills interleave with quarter-0 attention
  (first exp at ~18us instead of ~52us).
- Post phase: batched LN stats, rsqrt computed on DVE via the inverse-
  sqrt bit trick + 2 Newton steps (no scalar table switches), gelu reads
  PSUM directly (bias is zero), sin range reduction via the fused
  add_range_wrap DVE op, paired transpose evacuations, bf16 output DMA
  (host converts to f32).

Matmuls run in bf16 (fp32 PSUM accumulation) except the PV product, which
runs in fp8e4m3 with DoubleRow packing. Softmax skips the max-subtraction;
the denominator is a 65th ones-column appended to v. Wo bias is folded
into xres on the host; the FAN gate scale is fused into the residual add.
"""

import math

import numpy as np
import ml_dtypes

B, S, D, H, E = 2, 2048, 1024, 16, 64
P_DIM, G_DIM = 256, 512
SC = 512  # rows per core
NCORES = 8
LN_EPS = 1e-5

_bf = ml_dtypes.bfloat16
_f8 = ml_dtypes.float8_e4m3fn

_prog_cache = {}


def _build_program(
    gv: float, ln_triv=(False, False, False, False), bg_zero=True, bv_zero=True,
    dbg=False,
):
    from contextlib import ExitStack

    import concourse.bass as bass
    import concourse.bacc as bacc
    import concourse.tile as tile
    import concourse.mybir as mybir

    f32 = mybir.dt.float32
    i32 = mybir.dt.int32
    bf = mybir.dt.bfloat16
    f8 = mybir.dt.float8e4
    AF = mybir.ActivationFunctionType
    ALU = mybir.AluOpType

    nc = bacc.Bacc(
        "TRN2",
        target_bir_lowering=False,
        debug=False,
        enable_asserts=False,
        num_devices=NCORES,
    )

    # ---------------- DRAM I/O ----------------
    # xT / xTf8 arrive with per-core PERMUTED key-quarters: quarter 0 is the
    # core's own row chunk (so q-projection reuses the same load).
    d_xT = nc.dram_tensor("xT", [D, S], bf, kind="ExternalInput")
    # fp8 x^T pre-tiled host-side as [quarter, partition, kc, col] so each
    # quarter is one dense partition-major DMA
    d_xTf8 = nc.dram_tensor("xTf8", [4, 128, 8, 512], f8, kind="ExternalInput")
    d_xres = nc.dram_tensor("xres", [SC, D], bf, kind="ExternalInput")
    d_wqT = nc.dram_tensor("wqT", [D, D], bf, kind="ExternalInput")
    d_wkT = nc.dram_tensor("wkT", [D, D], bf, kind="ExternalInput")
    d_wvT = nc.dram_tensor("wvT", [D, D], f8, kind="ExternalInput")
    d_woT = nc.dram_tensor("woT", [D, D], f8, kind="ExternalInput")
    d_wpT = nc.dram_tensor("wpT", [D, P_DIM], bf, kind="ExternalInput")
    d_wgT = nc.dram_tensor("wgT", [D, G_DIM], bf, kind="ExternalInput")
    d_bqc = nc.dram_tensor("bqc", [128, 8], f32, kind="ExternalInput")
    d_bkc = nc.dram_tensor("bkc", [128, 8], f32, kind="ExternalInput")
    d_bvf = nc.dram_tensor("bvf", [D], f32, kind="ExternalInput")
    d_bgf = nc.dram_tensor("bgf", [G_DIM], f32, kind="ExternalInput")
    d_ln1w = nc.dram_tensor("ln1w", [D], f32, kind="ExternalInput")
    d_ln1b = nc.dram_tensor("ln1b", [D], f32, kind="ExternalInput")
    d_ln2w = nc.dram_tensor("ln2w", [D], f32, kind="ExternalInput")
    d_ln2b = nc.dram_tensor("ln2b", [D], f32, kind="ExternalInput")
    d_offs2 = nc.dram_tensor("offs2", [P_DIM * 2], f32, kind="ExternalInput")
    d_sel = nc.dram_tensor("sel", [8, 2, 16, 64], bf, kind="ExternalInput")
    d_ident = nc.dram_tensor("ident", [128, 128], bf, kind="ExternalInput")
    d_out = nc.dram_tensor("out", [SC, D], bf, kind="ExternalOutput")
    if dbg:
        d_dz = nc.dram_tensor("dz", [128, 4, D], f32, kind="ExternalOutput")
        d_dy = nc.dram_tensor("dy", [128, 4, D], bf, kind="ExternalOutput")
        d_drsd = nc.dram_tensor("drsd", [128, 4, 3], f32, kind="ExternalOutput")
        d_dtarg = nc.dram_tensor("dtarg", [128, 4, 512], f32, kind="ExternalOutput")
        d_dg = nc.dram_tensor("dg", [128, 4, 512], f32, kind="ExternalOutput")
        d_dz2 = nc.dram_tensor("dz2", [128, 4, D], bf, kind="ExternalOutput")
        d_draw = nc.dram_tensor("draw", [128, 16, 512], bf, kind="ExternalOutput")
        d_dqt = nc.dram_tensor("dqt", [128, 8, SC], bf, kind="ExternalOutput")
        d_dkt = nc.dram_tensor("dkt", [128, 8, S], bf, kind="ExternalOutput")
        d_dva = nc.dram_tensor("dva", [128, 8, 2, 16, 80], mybir.dt.float8e4,
                               kind="ExternalOutput")

    def bcast(handle, parts):
        ap_ = handle.ap()
        return bass.AP(
            tensor=ap_.tensor, offset=ap_.offset, ap=[[0, parts]] + list(ap_.ap)
        )

    def chunked(handle, nck, cols):
        ap_ = handle.ap()
        return bass.AP(
            tensor=ap_.tensor,
            offset=ap_.offset,
            ap=[[cols, 128], [128 * cols, nck], [1, cols]],
        )

    with tile.TileContext(nc, pool_alloc_mode="queue") as tc:
        with ExitStack() as ctx:
            # ---- PE warmup: junk matmuls to release the HAM clock gate ----
            warm = tc.alloc_tile_pool(name="warm", bufs=1)
            wjunk = warm.tile([128, 640], bf)
            nc.vector.memset(wjunk, 0.0)
            with tc.tile_pool(name="pwm", bufs=1, space="PSUM") as pwm:
                pswm = pwm.tile([128, 512], f32, name="pswm")
                for _ in range(20):
                    nc.tensor.matmul(
                        pswm,
                        lhsT=wjunk[:, 0:128],
                        rhs=wjunk[:, 128:640],
                        start=True,
                        stop=True,
                    )
            warm.release()

            misc1 = tc.alloc_tile_pool(name="misc1", bufs=1)

            bqc_sb = misc1.tile([128, 8], f32)
            nc.gpsimd.dma_start(out=bqc_sb, in_=d_bqc.ap())
            bkc_sb = misc1.tile([128, 8], f32)
            nc.gpsimd.dma_start(out=bkc_sb, in_=d_bkc.ap())
            eps_sb = misc1.tile([128, 1], f32)
            nc.vector.memset(eps_sb, LN_EPS)
            bv_bc = None
            if not bv_zero:
                bv_bc = misc1.tile([128, D], f32)
                nc.gpsimd.dma_start(out=bv_bc, in_=bcast(d_bvf, 128))

            # persistent attention tiles (right side)
            kv = tc.alloc_tile_pool(name="kv", bufs=1, side="right")
            qT_sb = kv.tile([128, 8, SC], bf)
            kT_sb = kv.tile([128, 8, S], bf)
            # v in fp8, DoubleRow layout: [keys, tb-pair, parity, head, 80pad]
            vaug = kv.tile([128, 8, 2, 16, 80], f8)
            nc.vector.memset(vaug[:, :, :, :, 64:65], 1.0)

            # raw attention output staging (lives through normalize)
            apo = tc.alloc_tile_pool(name="attnp", bufs=1, side="right")
            raw_sb = apo.tile([128, 16, 512], bf)
            # [8 heads-in-batch, 2 batches, 512 rows] layout keeps every
            # engine access at partition base 0
            den16 = apo.tile([8, 2, 512], bf)
            rec16 = apo.tile([8, 2, 512], bf)

            # weights + x^T (live through attention; right side, freed after)
            qkvw2 = tc.alloc_tile_pool(name="qkvw2", bufs=1, side="right")
            wk_sb = qkvw2.tile([128, 8, D], bf)
            wv_sb = qkvw2.tile([128, 8, D], f8)
            xt_sb = qkvw2.tile([128, 8, S], bf)
            xt_f8 = qkvw2.tile([128, 8, S], f8)

            # q-projection operands (left, freed before the post phase)
            qkvw = tc.alloc_tile_pool(name="qkvw", bufs=1)
            wq_sb = qkvw.tile([128, 8, D], bf)

            def xt_quarter_ap(dram, tb):
                ap_ = dram.ap()
                return bass.AP(
                    tensor=ap_.tensor,
                    offset=tb * 512,
                    ap=[[S, 128], [128 * S, 8], [1, 512]],
                )

            with tc.high_priority():
                # scalar queue: wk chunks (scalar is free until first exp)
                for kc in range(8):
                    nc.scalar.dma_start(
                        out=wk_sb[:, kc, :],
                        in_=d_wkT.ap()[kc * 128 : (kc + 1) * 128, :],
                    )
                # sync queue: wq chunks, then quarters 1-3 of x (not needed
                # until attention quarter 1)
                for kc in range(8):
                    nc.sync.dma_start(
                        out=wq_sb[:, kc, :],
                        in_=d_wqT.ap()[kc * 128 : (kc + 1) * 128, :],
                    )
                for tb in range(1, 4):
                    nc.sync.dma_start(
                        out=xt_sb[:, :, tb * 512 : (tb + 1) * 512],
                        in_=xt_quarter_ap(d_xT, tb),
                    )
                    nc.sync.dma_start(
                        out=xt_f8[:, :, tb * 512 : (tb + 1) * 512],
                        in_=d_xTf8.ap()[tb],
                    )
                # gpsimd queue: quarter 0 of x (bf16+fp8) and wv only
                nc.gpsimd.dma_start(
                    out=xt_sb[:, :, 0:512], in_=xt_quarter_ap(d_xT, 0)
                )
                nc.gpsimd.dma_start(out=wv_sb, in_=chunked(d_wvT, 8, D))
                nc.gpsimd.dma_start(
                    out=xt_f8[:, :, 0:512], in_=d_xTf8.ap()[0]
                )

            # ================= attention (k/v fills interleaved) =================
            fil_stack = ExitStack()
            with tc.tile_pool(name="ppa", bufs=1, space="PSUM") as ppa:
                fil = fil_stack.enter_context(
                    tc.tile_pool(name="fil", bufs=1, space="PSUM")
                )

                def emit_kt_block(tb, m):
                    ps = fil.tile([128, 512], f32, tag="fp", bufs=2, name="fp")
                    for kc in range(8):
                        nc.tensor.matmul(
                            ps,
                            lhsT=wk_sb[:, kc, m * 128 : (m + 1) * 128],
                            rhs=xt_sb[:, kc, tb * 512 : (tb + 1) * 512],
                            start=(kc == 0),
                            stop=(kc == 7),
                        )
                    nc.vector.tensor_scalar(
                        out=kT_sb[:, m, tb * 512 : (tb + 1) * 512],
                        in0=ps,
                        scalar1=bkc_sb[:, m : m + 1],
                        scalar2=None,
                        op0=ALU.add,
                    )

                def emit_v_block(tb, tm, h2):
                    tcx = tb * 4 + tm  # 128-row key block index (0..15)
                    ps = fil.tile([128, 512], f32, tag="fp", bufs=2, name="fp")
                    for c in range(4):
                        nc.tensor.matmul(
                            ps,
                            lhsT=xt_f8[
                                :, 2 * c : 2 * c + 2,
                                tb * 512 + tm * 128 : tb * 512 + (tm + 1) * 128,
                            ],
                            rhs=wv_sb[:, 2 * c : 2 * c + 2, h2 * 512 : (h2 + 1) * 512],
                            start=(c == 0),
                            stop=(c == 3),
                            perf_mode=mybir.MatmulPerfMode.DoubleRow,
                        )
                    v_out = vaug[:, tcx // 2, tcx % 2, h2 * 8 : (h2 + 1) * 8, 0:64]
                    if bv_zero:
                        nc.vector.tensor_copy(v_out, ps)
                    else:
                        nc.vector.tensor_tensor(
                            out=v_out,
                            in0=ps,
                            in1=bv_bc[:, h2 * 512 : (h2 + 1) * 512],
                            op=ALU.add,
                        )

                def quarter_fills(q):
                    """16 fill thunks that produce k^T and v for quarter q."""
                    fns = []
                    for m in range(8):
                        fns.append(lambda m=m: emit_kt_block(q, m))
                    for tm in range(4):
                        for h2 in range(2):
                            fns.append(lambda tm=tm, h2=h2: emit_v_block(q, tm, h2))
                    # interleave k and v emissions
                    out = []
                    for a, b2 in zip(fns[:8], fns[8:]):
                        out.append(a)
                        out.append(b2)
                    return out

                def attn_pair(p, q, fills):
                    opsums = []
                    for j in range(2):
                        op = ppa.tile(
                            [65, 512], f32, tag=f"opsum{j}", bufs=1, name=f"opsum{j}"
                        )
                        opsums.append(op)
                    for tp in range(2):
                        tbp = q * 2 + tp
                        probs = apo.tile(
                            [128, 2, 2, 512], f8, tag="probs", bufs=4, name="probs"
                        )
                        for parity in range(2):
                            tb = q * 4 + tp * 2 + parity
                            if fills:
                                fills.pop(0)()
                            ps2 = ppa.tile(
                                [128, 2, 512], f32, tag="ps2", bufs=2, name="ps2"
                            )
                            for j in range(2):
                                off = j * 64
                                nc.tensor.matmul(
                                    ps2[:, j, :],
                                    lhsT=kT_sb[
                                        off : off + 64, p, tb * 128 : (tb + 1) * 128
                                    ],
                                    rhs=qT_sb[off : off + 64, p, :],
                                    start=True,
                                    stop=True,
                                )
                            nc.scalar.activation(
                                out=probs[:, parity, :, :],
                                in_=ps2,
                                func=AF.Exp,
                                scale=1.0 / math.sqrt(E),
                            )
                        for j in range(2):
                            nc.tensor.matmul(
                                opsums[j],
                                lhsT=vaug[:, tbp, :, 2 * p + j, 0:65],
                                rhs=probs[:, :, j, :],
                                start=(tp == 0),
                                stop=(tp == 1),
                                perf_mode=mybir.MatmulPerfMode.DoubleRow,
                            )
                    for j in range(2):
                        if q == 0:
                            nc.vector.tensor_copy(
                                raw_sb[0:65, 2 * p + j, :], opsums[j]
                            )
                        else:
                            nc.vector.tensor_tensor(
                                out=raw_sb[0:65, 2 * p + j, :],
                                in0=opsums[j],
                                in1=raw_sb[0:65, 2 * p + j, :],
                                op=ALU.add,
                            )

                def emit_q_block(m):
                    """q^T chunk m through the same fill PSUM pool."""
                    ps = fil.tile([128, 512], f32, tag="fp", bufs=2, name="fp")
                    for kc in range(8):
                        nc.tensor.matmul(
                            ps,
                            lhsT=wq_sb[:, kc, m * 128 : (m + 1) * 128],
                            rhs=xt_sb[:, kc, 0:512],
                            start=(kc == 0),
                            stop=(kc == 7),
                        )
                    nc.vector.tensor_scalar(
                        out=qT_sb[:, m, :],
                        in0=ps,
                        scalar1=bqc_sb[:, m : m + 1],
                        scalar2=None,
                        op0=ALU.add,
                    )

                # quarter 0: q-proj + fills interleaved with attention pairs
                qm = [lambda m=m: emit_q_block(m) for m in range(8)]
                kt = [lambda m=m: emit_kt_block(0, m) for m in range(8)]
                vb = {
                    (tm, h2): (lambda tm=tm, h2=h2: emit_v_block(0, tm, h2))
                    for tm in range(4)
                    for h2 in range(2)
                }
                # prefix: everything pair 0 needs
                for f in (
                    qm[0], kt[0],
                    vb[(0, 0)], vb[(1, 0)], vb[(2, 0)], vb[(3, 0)],
                ):
                    f()
                fills_q0 = [
                    qm[1], kt[1], qm[2], kt[2], qm[3], kt[3],
                    qm[4], kt[4], vb[(0, 1)], qm[5],
                    kt[5], vb[(1, 1)], qm[6], kt[6],
                    vb[(2, 1)], qm[7], kt[7], vb[(3, 1)],
                ] + quarter_fills(1)
                for p in range(8):
                    attn_pair(p, 0, fills_q0)
                while fills_q0:
                    fills_q0.pop(0)()
                qkvw.release()

                fills2 = quarter_fills(2)
                for p in range(8):
                    attn_pair(p, 1, fills2)
                while fills2:
                    fills2.pop(0)()

                # ---- post-phase weights: DMA them only after quarter 1's
                # attention has produced data (tiny copies create the dep) ----
                misc2 = tc.alloc_tile_pool(name="misc2", bufs=1)
                ln1w_bc = ln1b_bc = None
                if not (ln_triv[0] and ln_triv[1]):
                    ln1w_bc = misc2.tile([128, D], f32)
                    nc.gpsimd.dma_start(out=ln1w_bc, in_=bcast(d_ln1w, 128))
                    ln1b_bc = misc2.tile([128, D], f32)
                    nc.gpsimd.dma_start(out=ln1b_bc, in_=bcast(d_ln1b, 128))
                ln2w_bc = ln2b_bc = None
                if not (ln_triv[2] and ln_triv[3]):
                    ln2w_bc = misc2.tile([128, D], f32)
                    nc.gpsimd.dma_start(out=ln2w_bc, in_=bcast(d_ln2w, 128))
                    ln2b_bc = misc2.tile([128, D], f32)
                    nc.gpsimd.dma_start(out=ln2b_bc, in_=bcast(d_ln2b, 128))
                bg_bc = None
                if not bg_zero:
                    bg_bc = misc2.tile([128, G_DIM], f32)
                    nc.gpsimd.dma_start(out=bg_bc, in_=bcast(d_bgf, 128))
                offs2_bc = misc2.tile([128, P_DIM * 2], f32)
                nc.gpsimd.dma_start(out=offs2_bc, in_=bcast(d_offs2, 128))
                sel_sb = misc2.tile([8, 2, 16, 64], bf)
                nc.gpsimd.dma_start(out=sel_sb, in_=d_sel.ap())
                ident_sb = misc2.tile([128, 128], bf)
                nc.gpsimd.dma_start(out=ident_sb, in_=d_ident.ap())
                xres_sb = misc2.tile([128, 4, D], bf)
                wo_sb = misc2.tile([128, 8, D], f8)
                # delay dep: read a write-once cell produced by quarter-2's
                # k fill (emitted during quarter 1) -- no WAR on hot tiles
                for big in (xres_sb, wo_sb):
                    nc.vector.tensor_copy(
                        big[0:1, 0, 0:1], kT_sb[0:1, 0, 1024:1025]
                    )
                nc.sync.dma_start(
                    out=xres_sb,
                    in_=bass.AP(
                        tensor=d_xres.ap().tensor,
                        offset=0,
                        ap=[[D, 128], [128 * D, 4], [1, D]],
                    ),
                )
                nc.sync.dma_start(out=wo_sb, in_=chunked(d_woT, 8, D))
                attn_oT = misc2.tile([128, 8, SC], f8)
                odd_sb = misc2.tile([64, 8, 512], f8)
                den32 = apo.tile([8, 2, 512], f32, name="den32")
                rec32 = apo.tile([8, 2, 512], f32, name="rec32")

                fills3 = quarter_fills(3)
                for p in range(8):
                    attn_pair(p, 2, fills3)
                while fills3:
                    fills3.pop(0)()

                fil_stack.close()  # free fill PSUM banks for the normalize

                # quarter 3: fused per-batch normalize in the attention tail
                with tc.tile_pool(name="ppn", bufs=2, space="PSUM") as ppn:

                    def normalize_batch(b):
                        hs = slice(8 * b, 8 * b + 8)
                        nc.sync.dma_start(
                            out=den16[:, b, :], in_=raw_sb[64:65, hs, :]
                        )
                        nc.vector.tensor_copy(den32[:, b, :], den16[:, b, :])
                        nc.vector.reciprocal_approx_fast(
                            out=rec32[:, b, :], in_=den32[:, b, :]
                        )
                        nc.vector.tensor_copy(rec16[:, b, :], rec32[:, b, :])
                        for h in range(8 * b, 8 * b + 8):
                            p_, j = h // 2, h % 2
                            div = ppn.tile([64, 512], f32, tag="div", name="div")
                            nc.tensor.matmul(
                                div,
                                lhsT=sel_sb[:, b, h, :],
                                rhs=rec16[:, b, :],
                                start=True,
                                stop=True,
                            )
                            if j == 0:
                                out_ap = attn_oT[0:64, p_, :]
                            else:
                                out_ap = odd_sb[0:64, p_, :]
                            nc.vector.tensor_tensor(
                                out=out_ap, in0=raw_sb[0:64, h, :],
                                in1=div, op=ALU.mult,
                            )

                    for p in range(8):
                        attn_pair(p, 3, [])
                        if p == 3:
                            normalize_batch(0)
                            nc.sync.dma_start(
                                out=attn_oT[64:128, 0:4, :], in_=odd_sb[:, 0:4, :]
                            )
                    if dbg:
                        nc.gpsimd.dma_start(out=d_draw.ap(), in_=raw_sb)
                    normalize_batch(1)
                nc.sync.dma_start(
                    out=attn_oT[64:128, 4:8, :], in_=odd_sb[:, 4:8, :]
                )
            qkvw2.release()
            if dbg:
                nc.gpsimd.dma_start(out=d_dqt.ap(), in_=qT_sb)
                nc.gpsimd.dma_start(out=d_dkt.ap(), in_=kT_sb)
                nc.gpsimd.dma_start(out=d_dva.ap(), in_=vaug)
            apo.release()
            kv.release()

            # ================= Wo + LN1 + FAN + LN2 =================
            with tc.tile_pool(name="pw", bufs=1) as pw, tc.tile_pool(
                name="post", bufs=2
            ) as po, tc.tile_pool(name="ppp", bufs=2, space="PSUM") as ppp:
                wp_sb = pw.tile([128, 8, P_DIM], bf)
                nc.sync.dma_start(out=wp_sb, in_=chunked(d_wpT, 8, P_DIM))
                wg_sb = pw.tile([128, 8, G_DIM], bf)
                nc.sync.dma_start(out=wg_sb, in_=chunked(d_wgT, 8, G_DIM))
                z_sb = pw.tile([128, 4, D], f32, tag="zfan", name="z_sb")
                y_sb = pw.tile([128, 4, D], bf)
                yT_sb = pw.tile([128, 8, SC], bf)
                targ_sb = pw.tile([128, 4, 512], f32)
                # col 512 is a pad cell used to order the single Gelu after
                # the last Sin
                g_sb = pw.tile([128, 4, 513], f32)
                nc.vector.memset(g_sb[:, :, 512:513], 0.0)
                z2_sb = pw.tile([128, 4, D], bf, tag="z2b", name="z2_sb")
                stats2 = pw.tile([128, 4, 2, 6], f32, tag="st2", name="stats2")
                mv2 = pw.tile([128, 4, 2], f32, tag="mv2", name="mv2")

                stats1 = pw.tile([128, 4, 2, 6], f32, tag="st1", name="stats1")
                mv1 = pw.tile([128, 4, 2], f32, tag="mv1", name="mv1")

                def ln_apply(z_ap, mv_sc, rsd_sc, w_bc, b_bc, out_ap, w_triv, b_triv, tag):
                    if w_triv and b_triv:
                        nc.vector.tensor_scalar(
                            out=out_ap, in0=z_ap,
                            scalar1=mv_sc, scalar2=rsd_sc,
                            op0=ALU.subtract, op1=ALU.mult,
                        )
                        return
                    tmp = po.tile([128, D], f32, tag=f"lntmp{tag}", name="lntmp")
                    nc.vector.tensor_scalar(
                        out=tmp, in0=z_ap, scalar1=mv_sc, scalar2=rsd_sc,
                        op0=ALU.subtract, op1=ALU.mult,
                    )
                    if b_triv:
                        nc.vector.tensor_tensor(out=out_ap, in0=tmp, in1=w_bc, op=ALU.mult)
                        return
                    if not w_triv:
                        nc.vector.tensor_tensor(out=tmp, in0=tmp, in1=w_bc, op=ALU.mult)
                    nc.vector.tensor_tensor(out=out_ap, in0=tmp, in1=b_bc, op=ALU.add)

                # ---- Wo -> z -> LN1 stats per chunk; one batched sqrt ----
                for sc in range(4):
                    for h2 in range(2):
                        ps = ppp.tile([128, 512], f32, tag="wops", name="wops")
                        for c in range(4):
                            nc.tensor.matmul(
                                ps,
                                lhsT=attn_oT[
                                    :, 2 * c : 2 * c + 2, sc * 128 : (sc + 1) * 128
                                ],
                                rhs=wo_sb[
                                    :, 2 * c : 2 * c + 2, h2 * 512 : (h2 + 1) * 512
                                ],
                                start=(c == 0),
                                stop=(c == 3),
                                perf_mode=mybir.MatmulPerfMode.DoubleRow,
                            )
                        nc.vector.tensor_tensor(
                            out=z_sb[:, sc, h2 * 512 : (h2 + 1) * 512],
                            in0=ps,
                            in1=xres_sb[:, sc, h2 * 512 : (h2 + 1) * 512],
                            op=ALU.add,
                        )
                        nc.vector.bn_stats(
                            out=stats1[:, sc, h2, :],
                            in_=z_sb[:, sc, h2 * 512 : (h2 + 1) * 512],
                        )
                    nc.vector.bn_aggr(out=mv1[:, sc, :], in_=stats1[:, sc, :, :])
                sd1 = pw.tile([128, 2, 4], f32, tag="sd1", name="sd1")
                nc.scalar.activation(
                    out=sd1[:, 0, :], in_=mv1[:, :, 1], func=AF.Sqrt, bias=eps_sb
                )
                rsd1 = sd1[:, 1, :]
                nc.vector.reciprocal(rsd1, sd1[:, 0, :])
                if dbg:
                    nc.gpsimd.dma_start(out=d_drsd.ap()[:, :, 0:2], in_=mv1)
                    nc.gpsimd.dma_start(out=d_drsd.ap()[:, :, 2], in_=rsd1)
                for sc in range(4):
                    ln_apply(
                        z_sb[:, sc, :], mv1[:, sc, 0:1], rsd1[:, sc : sc + 1],
                        ln1w_bc, ln1b_bc, y_sb[:, sc, :],
                        ln_triv[0], ln_triv[1], "a",
                    )
                    # transpose y chunk -> yT (evacuations on the idle ScalarE)
                    for dc in range(8):
                        tp = ppp.tile([128, 128], bf, tag="tp", name="tp")
                        nc.tensor.transpose(
                            tp, y_sb[:, sc, dc * 128 : (dc + 1) * 128], ident_sb
                        )
                        nc.scalar.copy(
                            out=yT_sb[:, dc, sc * 128 : (sc + 1) * 128], in_=tp
                        )
                    # Wp -> sin args with fused range wrap
                    psp = ppp.tile([128, P_DIM], f32, tag="pps", name="pps")
                    for kc in range(8):
                        nc.tensor.matmul(
                            psp,
                            lhsT=yT_sb[:, kc, sc * 128 : (sc + 1) * 128],
                            rhs=wp_sb[:, kc, :],
                            start=(kc == 0),
                            stop=(kc == 7),
                        )
                    nc.vector.tensor_tensor(
                        out=targ_sb[:, sc, 0:256], in0=psp,
                        in1=offs2_bc[:, 0:256], op=ALU.add,
                    )
                    nc.vector.tensor_tensor(
                        out=targ_sb[:, sc, 256:512], in0=psp,
                        in1=offs2_bc[:, 256:512], op=ALU.add,
                    )
                    nc.vector.add_range_wrap(
                        out=targ_sb[:, sc, :], in_=targ_sb[:, sc, :],
                        shift=0.0, bound=math.pi, period=2.0 * math.pi,
                    )
                    # FAN gelu branch matmuls in the same per-chunk pipeline
                    psg = ppp.tile([128, G_DIM], f32, tag="ppg", name="ppg")
                    for kc in range(8):
                        nc.tensor.matmul(
                            psg,
                            lhsT=yT_sb[:, kc, sc * 128 : (sc + 1) * 128],
                            rhs=wg_sb[:, kc, :],
                            start=(kc == 0),
                            stop=(kc == 7),
                        )
                    if bg_zero:
                        nc.vector.tensor_copy(g_sb[:, sc, 0:512], psg)
                    else:
                        nc.vector.tensor_tensor(
                            out=g_sb[:, sc, 0:512], in0=psg, in1=bg_bc, op=ALU.add
                        )
                    # per-chunk Sin: sin-half of z2 + its stats flow early
                    nc.scalar.activation(
                        out=targ_sb[:, sc, :], in_=targ_sb[:, sc, :], func=AF.Sin
                    )
                    nc.vector.scalar_tensor_tensor(
                        out=z2_sb[:, sc, 0:512],
                        in0=targ_sb[:, sc, :],
                        scalar=float(gv),
                        in1=y_sb[:, sc, 0:512],
                        op0=ALU.mult,
                        op1=ALU.add,
                    )
                    nc.vector.bn_stats(
                        out=stats2[:, sc, 0, :], in_=z2_sb[:, sc, 0:512]
                    )
                # order the single Gelu after the last Sin via the pad cell
                nc.scalar.copy(
                    out=g_sb[0:1, 3, 512:513], in_=targ_sb[0:1, 3, 0:1]
                )
                g_flat = bass.AP(
                    tensor=g_sb.tensor, offset=g_sb.offset,
                    ap=[list(g_sb.ap[0]), [1, 4 * 513]],
                )
                nc.scalar.activation(out=g_flat, in_=g_flat, func=AF.Gelu)
                if dbg:
                    nc.gpsimd.dma_start(out=d_dy.ap(), in_=y_sb)
                    nc.gpsimd.dma_start(out=d_dtarg.ap(), in_=targ_sb)

                # gelu-half of z2 + stats, then batched sqrt + reciprocal
                for sc in range(4):
                    nc.vector.scalar_tensor_tensor(
                        out=z2_sb[:, sc, 512:1024],
                        in0=g_sb[:, sc, 0:512],
                        scalar=float(1.0 - gv),
                        in1=y_sb[:, sc, 512:1024],
                        op0=ALU.mult,
                        op1=ALU.add,
                    )
                    nc.vector.bn_stats(
                        out=stats2[:, sc, 1, :], in_=z2_sb[:, sc, 512:1024]
                    )
                    nc.vector.bn_aggr(out=mv2[:, sc, :], in_=stats2[:, sc, :, :])
                if dbg:
                    nc.gpsimd.dma_start(
                        out=d_dg.ap(), in_=g_sb[:, :, 0:512]
                    )
                    nc.gpsimd.dma_start(out=d_dz2.ap(), in_=z2_sb)
                    nc.gpsimd.dma_start(out=d_dz.ap(), in_=z_sb)
                sd2 = pw.tile([128, 2, 4], f32, tag="sd2", name="sd2")
                nc.scalar.activation(
                    out=sd2[:, 0, :], in_=mv2[:, :, 1], func=AF.Sqrt, bias=eps_sb
                )
                rsd2 = sd2[:, 1, :]
                nc.vector.reciprocal(rsd2, sd2[:, 0, :])
                for sc in range(4):
                    outt = po.tile([128, D], bf, tag="outt", name="outt")
                    ln_apply(
                        z2_sb[:, sc, :], mv2[:, sc, 0:1], rsd2[:, sc : sc + 1],
                        ln2w_bc, ln2b_bc, outt, ln_triv[2], ln_triv[3], sc % 2
                    )
                    nc.sync.dma_start(
                        out=d_out.ap()[sc * 128 : (sc + 1) * 128, :], in_=outt
                    )

            misc2.release()
            misc1.release()

    nc.compile()
    return nc


def _host_inputs(inputs):
    """Build the per-core in_maps (list of 8 dicts) plus baked gate value."""
    f32 = np.float32
    x = np.asarray(inputs["x"], f32)
    Wq = np.asarray(inputs["Wq"], f32)
    Wk = np.asarray(inputs["Wk"], f32)
    Wv = np.asarray(inputs["Wv"], f32)
    Wo = np.asarray(inputs["Wo"], f32)
    Wp = np.asarray(inputs["Wp"], f32)
    Wg = np.asarray(inputs["Wg"], f32)
    bq = np.asarray(inputs["bq"], f32)
    bk = np.asarray(inputs["bk"], f32)
    bv = np.asarray(inputs["bv"], f32)
    bo = np.asarray(inputs["bo"], f32)
    bp = np.asarray(inputs["bp"], f32)
    bg = np.asarray(inputs["bg"], f32)
    offset = np.asarray(inputs["offset"], f32)
    gate = np.asarray(inputs["gate"], f32)
    ln1_w = np.asarray(inputs["ln1_w"], f32)
    ln1_b = np.asarray(inputs["ln1_b"], f32)
    ln2_w = np.asarray(inputs["ln2_w"], f32)
    ln2_b = np.asarray(inputs["ln2_b"], f32)

    gv = float(1.0 / (1.0 + np.exp(-gate[0])))

    sel = np.zeros((8, 2, 16, 64), f32)
    for h in range(16):
        sel[h % 8, h // 8, h, :] = 1.0
    ident = np.eye(128, dtype=f32)

    shared = {
        "wqT": np.ascontiguousarray(Wq.T).astype(_bf),
        "wkT": np.ascontiguousarray(Wk.T).astype(_bf),
        "wvT": np.ascontiguousarray(Wv.T).astype(_f8),
        "woT": np.ascontiguousarray(Wo.T).astype(_f8),
        "wpT": np.ascontiguousarray(Wp.T).astype(_bf),
        "wgT": np.ascontiguousarray(Wg.T).astype(_bf),
        "bqc": np.ascontiguousarray(bq.reshape(8, 128).T),
        "bkc": np.ascontiguousarray(bk.reshape(8, 128).T),
        "bvf": bv,
        "bgf": bg,
        "ln1w": ln1_w,
        "ln1b": ln1_b,
        "ln2w": ln2_w,
        "ln2b": ln2_b,
        "offs2": np.concatenate([offset + bp, np.pi - offset + bp]).astype(f32),
        "sel": sel.astype(_bf),
        "ident": ident.astype(_bf),
    }

    in_maps = []
    xT_by_b = [np.ascontiguousarray(x[b].T).astype(_bf) for b in range(B)]
    xTf8_by_b = [t.astype(_f8) for t in xT_by_b]
    for c in range(NCORES):
        b, qc = c // 4, c % 4
        # key-quarters permuted so the core's own row chunk comes first
        perm = [qc] + [q for q in range(4) if q != qc]
        xT_p = np.ascontiguousarray(
            xT_by_b[b].reshape(D, 4, SC)[:, perm, :].reshape(D, S)
        )
        xTf8_p = xTf8_by_b[b].reshape(D, 4, SC)[:, perm, :].reshape(D, S)
        m = dict(shared)
        m["xT"] = xT_p
        m["xTf8"] = np.ascontiguousarray(
            xTf8_p.reshape(8, 128, 4, 512).transpose(2, 1, 0, 3)
        )
        m["xres"] = np.ascontiguousarray(
            (x[b, qc * SC : (qc + 1) * SC] + bo).astype(_bf)
        )
        in_maps.append(m)
    return in_maps, gv


def run(inputs, trace=False, tmpdir=None, dbg=False):
    """Run the kernel; returns (full_output, BassKernelResults)."""
    from concourse.bass_utils import run_bass_kernel_spmd

    in_maps, gv = _host_inputs(inputs)
    ln_triv = (
        bool(np.all(np.asarray(inputs["ln1_w"]) == 1.0)),
        bool(np.all(np.asarray(inputs["ln1_b"]) == 0.0)),
        bool(np.all(np.asarray(inputs["ln2_w"]) == 1.0)),
        bool(np.all(np.asarray(inputs["ln2_b"]) == 0.0)),
    )
    bg_zero = bool(np.all(np.asarray(inputs["bg"]) == 0.0))
    bv_zero = bool(np.all(np.asarray(inputs["bv"]) == 0.0))
    key = (round(gv, 9), ln_triv, bg_zero, bv_zero, dbg)
    if key not in _prog_cache:
        _prog_cache[key] = _build_program(gv, ln_triv, bg_zero, bv_zero, dbg)
    nc = _prog_cache[key]
    res = run_bass_kernel_spmd(
        nc, in_maps, core_ids=list(range(NCORES)), trace=trace, tmpdir=tmpdir
    )
    chunks = [np.asarray(res.results[c]["out"]).astype(np.float32) for c in range(NCORES)]
    full = np.concatenate(chunks, axis=0).reshape(B, S, D)
    return full, res


def kernel(**inputs) -> np.ndarray:
    out, _ = run(inputs, trace=False)
    return out
